# revision 25
# baseline (speedup 1.0000x reference)
"""MHA kernel for TRN2: B=4,T=2048,D=1024,H=16,HD=64 across 8 NeuronCores.

Sharding: core c -> batch c//2, query half c%2 (host rotates the sequence so
each core's queries are columns 0:1024 of x^T; softmax over keys is
permutation invariant). No collectives.

Design (HW ~441us vs 804us for the fp32r/PE-transpose baseline):
- All transposes happen on the host: x^T and per-group-packed W^T are fed
  directly, in bf16 (halves DMA, avoids the slow fp32 LDWEIGHTS/matmul path).
  All matmuls take bf16 operands and accumulate fp32 in PSUM.
- Logits for the two heads of a group run row-packed (K=64 tiles at
  partitions 0:64 / 64:128 -> concurrent on the PE array).
- A ones-column appended to V yields the softmax denominators in row 64 of
  the same PV accumulation. The normalize chain never touches PE or PSUM
  banks: DVE copy to SBUF (frees the bank), DMA shift of the sums row to
  partition 0, reciprocal_approx_fast, gpsimd partition_broadcast, DVE
  multiply, DMA partition-shift into the concat layout.
- Projection matmuls for the NEXT head group (and the V builds) are emitted
  as single-PSUM-slot filler units inside the softmax-bound attention loop,
  so the PE fills its exp-wait gaps; the scheduler interleaves them.
- Input DMA is split across both HWDGE queues (sync + scalar) and ordered so
  the first projections start as early as possible; a short dummy-matmul
  burst warms the PE clock (HAM) during the initial DMA wait.
"""
import sys
sys.path.insert(0, "/opt/trn_rl_repo")
import warnings
warnings.filterwarnings("ignore")

import numpy as np
import ml_dtypes
import concourse.bass as bass
import concourse.mybir as mybir
import concourse.tile as tile
from concourse import bacc
from concourse.bass_utils import run_bass_kernel_spmd

F32 = mybir.dt.float32
BF16 = mybir.dt.bfloat16
EXP = mybir.ActivationFunctionType.Exp

T, D = 2048, 1024
TQ = 1024          # queries per core
NG = 8             # head groups (2 heads each)
NSC = 16           # s chunks of 128
NDC = 8            # d chunks of 128
SCALE = 0.125      # 1/sqrt(64)


def build_nc():
    nc = bacc.Bacc("TRN2", target_bir_lowering=False, debug=False, num_devices=8)
    xt = nc.dram_tensor("xt", [D, T], BF16, kind="ExternalInput")       # x[b]^T rot
    wq = nc.dram_tensor("wq", [D, D], BF16, kind="ExternalInput")       # [g*128+p, (dc,hk)]
    wk = nc.dram_tensor("wk", [D, D], BF16, kind="ExternalInput")
    wv = nc.dram_tensor("wv", [256, 4096], BF16, kind="ExternalInput")  # [hh*128+p, (dc,c512)]
    wo = nc.dram_tensor("wo", [D, D], BF16, kind="ExternalInput")       # Wo^T
    bo = nc.dram_tensor("bo", [1, D], F32, kind="ExternalInput")
    y = nc.dram_tensor("y", [TQ, D], F32, kind="ExternalOutput")

    with tile.TileContext(nc) as tc:
        with (
            tc.tile_pool(name="persist", bufs=1) as pp,
            tc.tile_pool(name="xtp", bufs=1) as xp,
            tc.tile_pool(name="wqk", bufs=2) as wqkp,
            tc.tile_pool(name="wvp", bufs=1) as wvp,
            tc.tile_pool(name="wop", bufs=1) as wop,
            tc.tile_pool(name="qkt", bufs=2) as qktp,
            tc.tile_pool(name="vtp", bufs=2) as vtp,
            tc.tile_pool(name="small", bufs=2) as sp,
            tc.tile_pool(name="ptp", bufs=3) as ptp,
            tc.tile_pool(name="yp", bufs=2) as yp,
            tc.tile_pool(name="ps_work", bufs=2, space="PSUM") as psw,
            tc.tile_pool(name="ps_pv", bufs=2, space="PSUM") as psv,
            tc.tile_pool(name="ps_log", bufs=2, space="PSUM") as psl,
        ):
            # ---- group-0 weights first (small), then x^T halves across
            # both DMA queues so the first projections start early ----
            wq0 = wqkp.tile([128, NDC, 128], BF16, tag="wqT")
            wk0 = wqkp.tile([128, NDC, 128], BF16, tag="wkT")
            nc.sync.dma_start(out=wq0, in_=wq[0:128, :])
            nc.sync.dma_start(out=wk0, in_=wk[0:128, :])
            wvT = wvp.tile([128, NDC, 512], BF16, tag="wvT")
            nc.scalar.dma_start(out=wvT, in_=wv[0:128, :])
            xT = xp.tile([128, NDC, T], BF16, tag="xT")
            for th in range(2):
                for dc in range(NDC):
                    eng = nc.sync if dc % 2 == 0 else nc.scalar
                    eng.dma_start(
                        out=xT[:, dc, th * 1024:(th + 1) * 1024],
                        in_=xt[dc * 128:(dc + 1) * 128, th * 1024:(th + 1) * 1024])

            bias = pp.tile([128, D], F32)
            nc.sync.dma_start(
                out=bias, in_=bass.AP(tensor=bo, offset=0, ap=[[0, 128], [1, D]]))
            catT = pp.tile([128, NG, TQ], BF16, name="catT")
            woT = wop.tile([128, NG, D], BF16, tag="woT")
            for gg in range(NG):
                nc.scalar.dma_start(
                    out=woT[:, gg, :], in_=wo[gg * 128:(gg + 1) * 128, :])

            def load_wqk(g):
                wqT = wqkp.tile([128, NDC, 128], BF16, tag="wqT")
                wkT = wqkp.tile([128, NDC, 128], BF16, tag="wkT")
                nc.sync.dma_start(out=wqT, in_=wq[g * 128:(g + 1) * 128, :])
                nc.sync.dma_start(out=wkT, in_=wk[g * 128:(g + 1) * 128, :])
                return wqT, wkT

            def proj_unit(wT, dst, w):
                """One 512-wide output window; holds a single psw slot."""
                p = psw.tile([128, 512], F32, tag="work")
                for dc in range(NDC):
                    nc.tensor.matmul(
                        p, wT[:, dc, :], xT[:, dc, w * 512:(w + 1) * 512],
                        start=(dc == 0), stop=(dc == NDC - 1))
                nc.vector.tensor_copy(out=dst[:, w * 512:(w + 1) * 512], in_=p)

            def new_vhalf():
                """V for 8 heads -> [128 s, sc, 8 h, 65] (col 64 = ones:
                row 64 of the PV result = softmax sums)."""
                vt = vtp.tile([128, NSC, 8, 65], BF16, tag="vt")
                nc.vector.memset(vt[:, :, :, 64:65], 1.0)
                return vt

            def v_unit(vt, wvT, sc):
                p = psw.tile([128, 512], F32, tag="work")
                for dc in range(NDC):
                    nc.tensor.matmul(
                        p, xT[:, dc, sc * 128:(sc + 1) * 128], wvT[:, dc, :],
                        start=(dc == 0), stop=(dc == NDC - 1))
                nc.vector.tensor_copy(
                    out=vt[:, sc, :, 0:64],
                    in_=p.rearrange("p (h c) -> p h c", h=8))

            def attention(g, qt, kt, vt, fillers):
                fillers = list(fillers)
                fi = 0
                for qh in range(2):
                    qs = slice(qh * 512, (qh + 1) * 512)
                    pv0 = psv.tile([65, 512], F32, tag="pv")
                    pv1 = psv.tile([65, 512], F32, tag="pv")
                    j = 2 * (g % 4)
                    for sc in range(NSC):
                        lg = psl.tile([128, 2, 512], F32, tag="log")
                        nc.tensor.matmul(
                            lg[:, 0, :], kt[0:64, sc * 128:(sc + 1) * 128],
                            qt[0:64, qs], start=True, stop=True)
                        nc.tensor.matmul(
                            lg[:, 1, :], kt[64:128, sc * 128:(sc + 1) * 128],
                            qt[64:128, qs], start=True, stop=True)
                        pt = ptp.tile([128, 2, 512], BF16, tag="pt")
                        nc.scalar.activation(
                            out=pt.rearrange("p a b -> p (a b)"),
                            in_=lg.rearrange("p a b -> p (a b)"),
                            func=EXP, scale=SCALE)
                        nc.tensor.matmul(
                            pv0, vt[:, sc, j, :], pt[:, 0, :],
                            start=(sc == 0), stop=(sc == NSC - 1))
                        nc.tensor.matmul(
                            pv1, vt[:, sc, j + 1, :], pt[:, 1, :],
                            start=(sc == 0), stop=(sc == NSC - 1))
                        if fi < len(fillers) and (sc % 2 == 1 or len(fillers) - fi > 8):
                            fillers[fi]()
                            fi += 1
                    for hloc, pv in ((0, pv0), (1, pv1)):
                        # sums live in row 64. PE-free normalize: bf16 copy of
                        # the weights (frees the PSUM bank), fast reciprocal of
                        # the sums row, DMA shift to partition 0, gpsimd
                        # partition broadcast, DVE multiply (+DMA shift for the
                        # odd head's concat rows).
                        pvs = sp.tile([65, 512], BF16, tag="pvs")
                        with nc.allow_low_precision(
                                reason="softmax weights tolerate bf16"):
                            nc.vector.tensor_copy(out=pvs, in_=pv)
                        s0f = sp.tile([65, 512], F32, tag="s0f")
                        nc.vector.tensor_copy(
                            out=s0f[64:65, :], in_=pv[64:65, :])
                        rec1 = sp.tile([1, 512], F32, tag="rec1")
                        nc.sync.dma_start(out=rec1, in_=s0f[64:65, :])
                        rec1f = sp.tile([1, 512], F32, tag="rec1f")
                        # reciprocal_approx_fast only works at base partition 0
                        nc.vector.reciprocal_approx_fast(out=rec1f, in_=rec1)
                        rec = sp.tile([64, 512], F32, tag="rec")
                        nc.gpsimd.partition_broadcast(rec[:, :], rec1f[:, :])
                        if hloc == 0:
                            nc.vector.tensor_mul(
                                out=catT[0:64, g, qs], in0=pvs[0:64, :], in1=rec)
                        else:
                            tmp = sp.tile([64, 512], BF16, tag="tmp")
                            nc.vector.tensor_mul(
                                out=tmp, in0=pvs[0:64, :], in1=rec)
                            nc.sync.dma_start(
                                out=catT[64:128, g, qs], in_=tmp)
                while fi < len(fillers):
                    fillers[fi]()
                    fi += 1

            # ---- PE warm-up during the initial DMA wait: ~4us of dummy
            # matmuls release the HAM clock throttle before real work ----
            dummy = pp.tile([64, 512], BF16, name="dummy")
            nc.vector.memset(dummy, 0.5)
            dp = psw.tile([128, 512], F32, tag="work")
            for i in range(20):
                nc.tensor.matmul(dp[0:64, :], dummy[:, 0:64], dummy,
                                 start=(i == 0), stop=(i == 19))
            nc.vector.tensor_copy(out=dummy, in_=dp[0:64, :])

            # ---- prologue: QT/KT group 0 (needs only the q half of x^T),
            # then V half 0 ----
            qt = qktp.tile([128, TQ], BF16, tag="qt")
            kt = qktp.tile([128, T], BF16, tag="kt")
            proj_unit(wq0, qt, 0)
            proj_unit(wq0, qt, 1)
            proj_unit(wk0, kt, 0)
            proj_unit(wk0, kt, 1)
            vt = new_vhalf()
            for sc in range(NSC // 2):
                v_unit(vt, wvT, sc)
            proj_unit(wk0, kt, 2)
            proj_unit(wk0, kt, 3)
            for sc in range(NSC // 2, NSC):
                v_unit(vt, wvT, sc)

            vt_next = None
            for g in range(NG):
                fillers = []
                if g < NG - 1:
                    wqn, wkn = load_wqk(g + 1)
                    qt_n = qktp.tile([128, TQ], BF16, tag="qt")
                    kt_n = qktp.tile([128, T], BF16, tag="kt")
                    for w in range(2):
                        fillers.append(
                            lambda w_=wqn, d=qt_n, i=w: proj_unit(w_, d, i))
                    for w in range(4):
                        fillers.append(
                            lambda w_=wkn, d=kt_n, i=w: proj_unit(w_, d, i))
                if g == 1:
                    wvT = wvp.tile([128, NDC, 512], BF16, tag="wvT")
                    nc.sync.dma_start(out=wvT, in_=wv[128:256, :])
                    vt_next = new_vhalf()
                if g in (2, 3):
                    lo = 0 if g == 2 else NSC // 2
                    for sc in range(lo, lo + NSC // 2):
                        fillers.append(lambda v=vt_next, w=wvT, s=sc: v_unit(v, w, s))
                attention(g, qt, kt, vt, fillers)
                if g == 3:
                    vt = vt_next
                if g < NG - 1:
                    qt, kt = qt_n, kt_n

            # ---- final projection ----
            for qb in range(8):
                yt = yp.tile([128, D], F32, tag="yt")
                p0 = psw.tile([128, 512], F32, tag="work")
                p1 = psw.tile([128, 512], F32, tag="work")
                for gg in range(NG):
                    nc.tensor.matmul(
                        p0, catT[:, gg, qb * 128:(qb + 1) * 128],
                        woT[:, gg, 0:512], start=(gg == 0), stop=(gg == NG - 1))
                    nc.tensor.matmul(
                        p1, catT[:, gg, qb * 128:(qb + 1) * 128],
                        woT[:, gg, 512:1024], start=(gg == 0), stop=(gg == NG - 1))
                nc.vector.tensor_add(out=yt[:, 0:512], in0=p0, in1=bias[:, 0:512])
                nc.vector.tensor_add(out=yt[:, 512:1024], in0=p1, in1=bias[:, 512:1024])
                (nc.sync if qb % 2 == 0 else nc.scalar).dma_start(
                    out=y[qb * 128:(qb + 1) * 128, :], in_=yt)

    nc.compile()
    return nc


_CACHE = {}


def _make_in_maps(ins):
    bf = ml_dtypes.bfloat16
    x = np.asarray(ins["x"], dtype=np.float32)
    Wq2 = np.asarray(ins["Wq"], dtype=np.float32).reshape(D, D)
    Wk2 = np.asarray(ins["Wk"], dtype=np.float32).reshape(D, D)
    Wv2 = np.asarray(ins["Wv"], dtype=np.float32).reshape(D, D)
    Wo2 = np.asarray(ins["Wo"], dtype=np.float32)
    # per-group packed W^T: wq_r[g*128+p, dc*128+j] = Wq2[g*128+j, dc*128+p]
    wq_r = np.ascontiguousarray(
        Wq2.reshape(8, 128, 8, 128).transpose(0, 3, 2, 1).reshape(D, D).astype(bf))
    wk_r = np.ascontiguousarray(
        Wk2.reshape(8, 128, 8, 128).transpose(0, 3, 2, 1).reshape(D, D).astype(bf))
    # wv_r[hh*128+p, dc*512+c] = Wv2[hh*512+c, dc*128+p]
    wv_r = np.ascontiguousarray(
        Wv2.reshape(2, 512, 8, 128).transpose(0, 3, 2, 1).reshape(256, 4096).astype(bf))
    wo_r = np.ascontiguousarray(Wo2.T.astype(bf))
    bo2 = np.ascontiguousarray(
        np.asarray(ins["bo"], dtype=np.float32).reshape(1, D))
    xT = x.transpose(0, 2, 1).astype(bf)  # [4, 1024, 2048]
    in_maps = []
    for c in range(8):
        b, h = c // 2, c % 2
        if h == 0:
            xtc = np.ascontiguousarray(xT[b])
        else:
            xtc = np.ascontiguousarray(
                np.concatenate([xT[b][:, TQ:], xT[b][:, :TQ]], axis=1))
        in_maps.append({"xt": xtc, "wq": wq_r, "wk": wk_r, "wv": wv_r,
                        "wo": wo_r, "bo": bo2})
    return in_maps


def kernel(x, Wq, Wk, Wv, Wo, bo):
    if "nc" not in _CACHE:
        _CACHE["nc"] = build_nc()
    nc = _CACHE["nc"]
    in_maps = _make_in_maps(
        {"x": x, "Wq": Wq, "Wk": Wk, "Wv": Wv, "Wo": Wo, "bo": bo})
    res = run_bass_kernel_spmd(nc, in_maps, core_ids=list(range(8)))
    out = np.empty((4, T, D), dtype=np.float32)
    for c in range(8):
        b, h = c // 2, c % 2
        out[b, h * TQ:(h + 1) * TQ] = res.results[c]["y"]
    return out


# revision 26
# speedup vs baseline: 1.1410x; 1.1410x over previous
"""MHA kernel for TRN2: B=4,T=2048,D=1024,H=16,HD=64 across 8 NeuronCores.

Sharding: core c -> batch c//2, query half c%2 (host rotates the sequence so
each core's queries are columns 0:1024 of x^T; softmax over keys is
permutation invariant). No collectives.

Design (HW ~441us vs 804us for the fp32r/PE-transpose baseline):
- All transposes happen on the host: x^T and per-group-packed W^T are fed
  directly, in bf16 (halves DMA, avoids the slow fp32 LDWEIGHTS/matmul path).
  All matmuls take bf16 operands and accumulate fp32 in PSUM.
- Logits for the two heads of a group run row-packed (K=64 tiles at
  partitions 0:64 / 64:128 -> concurrent on the PE array).
- A ones-column appended to V yields the softmax denominators in row 64 of
  the same PV accumulation. The normalize chain never touches PE or PSUM
  banks: DVE copy to SBUF (frees the bank), DMA shift of the sums row to
  partition 0, reciprocal_approx_fast, gpsimd partition_broadcast, DVE
  multiply, DMA partition-shift into the concat layout.
- Projection matmuls for the NEXT head group (and the V builds) are emitted
  as single-PSUM-slot filler units inside the softmax-bound attention loop,
  so the PE fills its exp-wait gaps; the scheduler interleaves them.
- Input DMA is split across both HWDGE queues (sync + scalar) and ordered so
  the first projections start as early as possible; a short dummy-matmul
  burst warms the PE clock (HAM) during the initial DMA wait.
"""
import sys
sys.path.insert(0, "/opt/trn_rl_repo")
import warnings
warnings.filterwarnings("ignore")

import numpy as np
import ml_dtypes
import concourse.bass as bass
import concourse.mybir as mybir
import concourse.tile as tile
from concourse import bacc
from concourse.bass_utils import run_bass_kernel_spmd

F32 = mybir.dt.float32
BF16 = mybir.dt.bfloat16
EXP = mybir.ActivationFunctionType.Exp

T, D = 2048, 1024
TQ = 1024          # queries per core
NG = 8             # head groups (2 heads each)
NSC = 16           # s chunks of 128
NDC = 8            # d chunks of 128
SCALE = 0.125      # 1/sqrt(64)


def build_nc():
    nc = bacc.Bacc("TRN2", target_bir_lowering=False, debug=False, num_devices=8)
    xt = nc.dram_tensor("xt", [D, T], BF16, kind="ExternalInput")       # x[b]^T rot
    wq = nc.dram_tensor("wq", [D, D], BF16, kind="ExternalInput")       # [g*128+p, (dc,hk)]
    wk = nc.dram_tensor("wk", [D, D], BF16, kind="ExternalInput")
    wv = nc.dram_tensor("wv", [256, 4096], BF16, kind="ExternalInput")  # [hh*128+p, (dc,c512)]
    wo = nc.dram_tensor("wo", [D, D], BF16, kind="ExternalInput")       # Wo^T
    bo = nc.dram_tensor("bo", [1, D], F32, kind="ExternalInput")
    y = nc.dram_tensor("y", [TQ, D], F32, kind="ExternalOutput")

    with tile.TileContext(nc) as tc:
        with (
            tc.tile_pool(name="persist", bufs=1) as pp,
            tc.tile_pool(name="xtp", bufs=1) as xp,
            tc.tile_pool(name="wqk", bufs=2) as wqkp,
            tc.tile_pool(name="wvp", bufs=1) as wvp,
            tc.tile_pool(name="wop", bufs=1) as wop,
            tc.tile_pool(name="qkt", bufs=2) as qktp,
            tc.tile_pool(name="vtp", bufs=2) as vtp,
            tc.tile_pool(name="small", bufs=2) as sp,
            tc.tile_pool(name="ptp", bufs=3) as ptp,
            tc.tile_pool(name="yp", bufs=2) as yp,
            tc.tile_pool(name="ps_work", bufs=2, space="PSUM") as psw,
            tc.tile_pool(name="ps_pv", bufs=2, space="PSUM") as psv,
            tc.tile_pool(name="ps_log", bufs=2, space="PSUM") as psl,
        ):
            # ---- group-0 weights first (small), then x^T halves across
            # both DMA queues so the first projections start early ----
            wq0 = wqkp.tile([128, NDC, 128], BF16, tag="wqT")
            wk0 = wqkp.tile([128, NDC, 128], BF16, tag="wkT")
            nc.sync.dma_start(out=wq0, in_=wq[0:128, :])
            nc.sync.dma_start(out=wk0, in_=wk[0:128, :])
            xT = xp.tile([128, NDC, T], BF16, tag="xT")
            wvT = wvp.tile([128, NDC, 512], BF16, tag="wvT")
            for dc in range(NDC):
                eng = nc.sync if dc % 2 == 0 else nc.scalar
                eng.dma_start(out=xT[:, dc, 0:1024],
                              in_=xt[dc * 128:(dc + 1) * 128, 0:1024])
            nc.scalar.dma_start(out=wvT, in_=wv[0:128, :])
            for dc in range(NDC):
                eng = nc.sync if dc % 2 == 0 else nc.scalar
                eng.dma_start(out=xT[:, dc, 1024:2048],
                              in_=xt[dc * 128:(dc + 1) * 128, 1024:2048])

            bias = pp.tile([128, D], F32)
            nc.scalar.dma_start(
                out=bias, in_=bass.AP(tensor=bo, offset=0, ap=[[0, 128], [1, D]]))
            catT = pp.tile([128, NG, TQ], BF16, name="catT")
            woT = wop.tile([128, NG, D], BF16, tag="woT")
            for gg in range(NG):
                nc.scalar.dma_start(
                    out=woT[:, gg, :], in_=wo[gg * 128:(gg + 1) * 128, :])

            def load_wqk(g):
                wqT = wqkp.tile([128, NDC, 128], BF16, tag="wqT")
                wkT = wqkp.tile([128, NDC, 128], BF16, tag="wkT")
                nc.sync.dma_start(out=wqT, in_=wq[g * 128:(g + 1) * 128, :])
                nc.sync.dma_start(out=wkT, in_=wk[g * 128:(g + 1) * 128, :])
                return wqT, wkT

            def proj_unit(wT, dst, w):
                """One 512-wide output window; holds a single psw slot."""
                p = psw.tile([128, 512], F32, tag="work")
                for dc in range(NDC):
                    nc.tensor.matmul(
                        p, wT[:, dc, :], xT[:, dc, w * 512:(w + 1) * 512],
                        start=(dc == 0), stop=(dc == NDC - 1))
                nc.vector.tensor_copy(out=dst[:, w * 512:(w + 1) * 512], in_=p)

            def new_vhalf():
                """V for 8 heads -> [128 s, sc, 8 h, 65] (col 64 = ones:
                row 64 of the PV result = softmax sums)."""
                vt = vtp.tile([128, NSC, 8, 65], BF16, tag="vt")
                nc.vector.memset(vt[:, :, :, 64:65], 1.0)
                return vt

            def v_unit(vt, wvT, sc):
                p = psw.tile([128, 512], F32, tag="work")
                for dc in range(NDC):
                    nc.tensor.matmul(
                        p, xT[:, dc, sc * 128:(sc + 1) * 128], wvT[:, dc, :],
                        start=(dc == 0), stop=(dc == NDC - 1))
                nc.vector.tensor_copy(
                    out=vt[:, sc, :, 0:64],
                    in_=p.rearrange("p (h c) -> p h c", h=8))

            def attention(g, qt, kt, vt, fillers):
                fillers = list(fillers)
                fi = 0
                for qh in range(2):
                    qs = slice(qh * 512, (qh + 1) * 512)
                    pv0 = psv.tile([65, 512], F32, tag="pv")
                    pv1 = psv.tile([65, 512], F32, tag="pv")
                    j = 2 * (g % 4)
                    for sc in range(NSC):
                        lg = psl.tile([128, 2, 512], F32, tag="log")
                        nc.tensor.matmul(
                            lg[:, 0, :], kt[0:64, sc * 128:(sc + 1) * 128],
                            qt[0:64, qs], start=True, stop=True)
                        nc.tensor.matmul(
                            lg[:, 1, :], kt[64:128, sc * 128:(sc + 1) * 128],
                            qt[64:128, qs], start=True, stop=True)
                        pt = ptp.tile([128, 2, 512], BF16, tag="pt")
                        nc.scalar.activation(
                            out=pt.rearrange("p a b -> p (a b)"),
                            in_=lg.rearrange("p a b -> p (a b)"),
                            func=EXP, scale=SCALE)
                        nc.tensor.matmul(
                            pv0, vt[:, sc, j, :], pt[:, 0, :],
                            start=(sc == 0), stop=(sc == NSC - 1))
                        nc.tensor.matmul(
                            pv1, vt[:, sc, j + 1, :], pt[:, 1, :],
                            start=(sc == 0), stop=(sc == NSC - 1))
                        if fi < len(fillers) and (sc % 2 == 1 or len(fillers) - fi > 8):
                            fillers[fi]()
                            fi += 1
                    for hloc, pv in ((0, pv0), (1, pv1)):
                        # sums live in row 64. PE-free normalize: bf16 copy of
                        # the weights (frees the PSUM bank), fast reciprocal of
                        # the sums row, DMA shift to partition 0, gpsimd
                        # partition broadcast, DVE multiply (+DMA shift for the
                        # odd head's concat rows).
                        pvs = sp.tile([65, 512], BF16, tag="pvs")
                        with nc.allow_low_precision(
                                reason="softmax weights tolerate bf16"):
                            nc.vector.tensor_copy(out=pvs, in_=pv)
                        s0f = sp.tile([65, 512], F32, tag="s0f")
                        nc.vector.tensor_copy(
                            out=s0f[64:65, :], in_=pv[64:65, :])
                        rec1 = sp.tile([1, 512], F32, tag="rec1")
                        nc.sync.dma_start(out=rec1, in_=s0f[64:65, :])
                        rec1f = sp.tile([1, 512], F32, tag="rec1f")
                        # reciprocal_approx_fast only works at base partition 0
                        nc.vector.reciprocal_approx_fast(out=rec1f, in_=rec1)
                        rec = sp.tile([64, 512], F32, tag="rec")
                        nc.gpsimd.partition_broadcast(rec[:, :], rec1f[:, :])
                        if hloc == 0:
                            nc.vector.tensor_mul(
                                out=catT[0:64, g, qs], in0=pvs[0:64, :], in1=rec)
                        else:
                            tmp = sp.tile([64, 512], BF16, tag="tmp")
                            nc.vector.tensor_mul(
                                out=tmp, in0=pvs[0:64, :], in1=rec)
                            nc.sync.dma_start(
                                out=catT[64:128, g, qs], in_=tmp)
                while fi < len(fillers):
                    fillers[fi]()
                    fi += 1

            # ---- PE warm-up during the initial DMA wait: ~4us of dummy
            # matmuls release the HAM clock throttle before real work ----
            dummy = pp.tile([64, 512], BF16, name="dummy")
            nc.vector.memset(dummy, 0.5)
            dp = psw.tile([128, 512], F32, tag="work")
            for i in range(20):
                nc.tensor.matmul(dp[0:64, :], dummy[:, 0:64], dummy,
                                 start=(i == 0), stop=(i == 19))
            nc.vector.tensor_copy(out=dummy, in_=dp[0:64, :])

            # ---- prologue: QT/KT group 0 (needs only the q half of x^T),
            # then V half 0 ----
            qt = qktp.tile([128, TQ], BF16, tag="qt")
            kt = qktp.tile([128, T], BF16, tag="kt")
            proj_unit(wq0, qt, 0)
            proj_unit(wq0, qt, 1)
            proj_unit(wk0, kt, 0)
            proj_unit(wk0, kt, 1)
            vt = new_vhalf()
            for sc in range(NSC // 2):
                v_unit(vt, wvT, sc)
            proj_unit(wk0, kt, 2)
            proj_unit(wk0, kt, 3)
            for sc in range(NSC // 2, NSC):
                v_unit(vt, wvT, sc)

            vt_next = None
            for g in range(NG):
                fillers = []
                if g < NG - 1:
                    wqn, wkn = load_wqk(g + 1)
                    qt_n = qktp.tile([128, TQ], BF16, tag="qt")
                    kt_n = qktp.tile([128, T], BF16, tag="kt")
                    for w in range(2):
                        fillers.append(
                            lambda w_=wqn, d=qt_n, i=w: proj_unit(w_, d, i))
                    for w in range(4):
                        fillers.append(
                            lambda w_=wkn, d=kt_n, i=w: proj_unit(w_, d, i))
                if g == 1:
                    wvT = wvp.tile([128, NDC, 512], BF16, tag="wvT")
                    nc.sync.dma_start(out=wvT, in_=wv[128:256, :])
                    vt_next = new_vhalf()
                if g in (2, 3):
                    lo = 0 if g == 2 else NSC // 2
                    for sc in range(lo, lo + NSC // 2):
                        fillers.append(lambda v=vt_next, w=wvT, s=sc: v_unit(v, w, s))
                attention(g, qt, kt, vt, fillers)
                if g == 3:
                    vt = vt_next
                if g < NG - 1:
                    qt, kt = qt_n, kt_n

            # ---- final projection ----
            for qb in range(8):
                yt = yp.tile([128, D], F32, tag="yt")
                p0 = psw.tile([128, 512], F32, tag="work")
                p1 = psw.tile([128, 512], F32, tag="work")
                for gg in range(NG):
                    nc.tensor.matmul(
                        p0, catT[:, gg, qb * 128:(qb + 1) * 128],
                        woT[:, gg, 0:512], start=(gg == 0), stop=(gg == NG - 1))
                    nc.tensor.matmul(
                        p1, catT[:, gg, qb * 128:(qb + 1) * 128],
                        woT[:, gg, 512:1024], start=(gg == 0), stop=(gg == NG - 1))
                nc.vector.tensor_add(out=yt[:, 0:512], in0=p0, in1=bias[:, 0:512])
                nc.vector.tensor_add(out=yt[:, 512:1024], in0=p1, in1=bias[:, 512:1024])
                (nc.sync if qb % 2 == 0 else nc.scalar).dma_start(
                    out=y[qb * 128:(qb + 1) * 128, :], in_=yt)

    nc.compile()
    return nc


_CACHE = {}


def _make_in_maps(ins):
    bf = ml_dtypes.bfloat16
    x = np.asarray(ins["x"], dtype=np.float32)
    Wq2 = np.asarray(ins["Wq"], dtype=np.float32).reshape(D, D)
    Wk2 = np.asarray(ins["Wk"], dtype=np.float32).reshape(D, D)
    Wv2 = np.asarray(ins["Wv"], dtype=np.float32).reshape(D, D)
    Wo2 = np.asarray(ins["Wo"], dtype=np.float32)
    # per-group packed W^T: wq_r[g*128+p, dc*128+j] = Wq2[g*128+j, dc*128+p]
    wq_r = np.ascontiguousarray(
        Wq2.reshape(8, 128, 8, 128).transpose(0, 3, 2, 1).reshape(D, D).astype(bf))
    wk_r = np.ascontiguousarray(
        Wk2.reshape(8, 128, 8, 128).transpose(0, 3, 2, 1).reshape(D, D).astype(bf))
    # wv_r[hh*128+p, dc*512+c] = Wv2[hh*512+c, dc*128+p]
    wv_r = np.ascontiguousarray(
        Wv2.reshape(2, 512, 8, 128).transpose(0, 3, 2, 1).reshape(256, 4096).astype(bf))
    wo_r = np.ascontiguousarray(Wo2.T.astype(bf))
    bo2 = np.ascontiguousarray(
        np.asarray(ins["bo"], dtype=np.float32).reshape(1, D))
    xT = x.transpose(0, 2, 1).astype(bf)  # [4, 1024, 2048]
    in_maps = []
    for c in range(8):
        b, h = c // 2, c % 2
        if h == 0:
            xtc = np.ascontiguousarray(xT[b])
        else:
            xtc = np.ascontiguousarray(
                np.concatenate([xT[b][:, TQ:], xT[b][:, :TQ]], axis=1))
        in_maps.append({"xt": xtc, "wq": wq_r, "wk": wk_r, "wv": wv_r,
                        "wo": wo_r, "bo": bo2})
    return in_maps


def kernel(x, Wq, Wk, Wv, Wo, bo):
    if "nc" not in _CACHE:
        _CACHE["nc"] = build_nc()
    nc = _CACHE["nc"]
    in_maps = _make_in_maps(
        {"x": x, "Wq": Wq, "Wk": Wk, "Wv": Wv, "Wo": Wo, "bo": bo})
    res = run_bass_kernel_spmd(nc, in_maps, core_ids=list(range(8)))
    out = np.empty((4, T, D), dtype=np.float32)
    for c in range(8):
        b, h = c // 2, c % 2
        out[b, h * TQ:(h + 1) * TQ] = res.results[c]["y"]
    return out
